# revision 9
# baseline (speedup 1.0000x reference)
"""Batch triplet loss on 8 TRN2 NeuronCores — v6: lagged-consumer pipeline.

v5 structure (fp8 DoubleRow Gram + sq_j seed matmuls; ACT drains psum->f16
full-d2; DVE f16 running maxes) with one key scheduling change: consumers are
EMITTED several tiles behind their producers, so their cross-engine semaphore
waits are already satisfied when reached (measured ~2-3us per unsatisfied
cross-engine wait on this hardware, ~0 when pre-satisfied):
  - ACT drain of psum tile i is emitted after the matmuls of tile i+DRAIN_LAG
  - DVE maxes for drained tile j are emitted after drain j+TT_LAG
"""

import os
from contextlib import ExitStack

import ml_dtypes
import numpy as np

import concourse.bass as bass
import concourse.tile as tile
from concourse import bacc, bass_utils, mybir

N = 8192
D = 1024
NCORES = 8
OWN = N // NCORES       # 1024
KT = D // 128           # 8
KK = KT // 2            # 4
JW = 512
DW = 1024
NPAN = 5
MOVW = NPAN * OWN       # 5120
NDS = MOVW // DW        # 5
IT = OWN // 128         # 8
ND2 = 3
EPS = 1e-6
MARGIN = 0.5

F8 = mybir.dt.float8e4
F16 = mybir.dt.float16
F32 = mybir.dt.float32

_NC = None

DS_GROUPS = [[0, 1], [2, 3], [4]]
DRAIN_LAG = int(os.environ.get("K_DRAINLAG", "2"))
TT_LAG = int(os.environ.get("K_TTLAG", "6"))


def _build_nc():
    REPEAT = int(os.environ.get("KBENCH_REPEAT", "1"))
    nc = bacc.Bacc("TRN2", target_bir_lowering=False, debug=False)
    mov = nc.dram_tensor("mov", [D, MOVW], F8, kind="ExternalInput").ap()
    sqi_d = nc.dram_tensor("sqi", [128, IT], F32, kind="ExternalInput").ap()
    sqr_d = nc.dram_tensor("sqr", [1, MOVW], F16, kind="ExternalInput").ap()
    out_m1 = nc.dram_tensor("out_m1", [128, IT * DW], F16, kind="ExternalOutput").ap()
    out_m2 = nc.dram_tensor("out_m2", [128, ND2 * DW], F16, kind="ExternalOutput").ap()

    mov_v = mov.rearrange("(k p) w -> p k w", p=128)

    with ExitStack() as ctx:
        tc = ctx.enter_context(tile.TileContext(nc))
        big = ctx.enter_context(tc.tile_pool(name="big", bufs=2))
        srp = ctx.enter_context(tc.tile_pool(name="srp", bufs=2))
        sqp = ctx.enter_context(tc.tile_pool(name="sqp", bufs=2))
        const = ctx.enter_context(tc.tile_pool(name="const", bufs=1))
        ttp = ctx.enter_context(tc.tile_pool(name="ttp", bufs=TT_LAG + 4))
        mcp = ctx.enter_context(tc.tile_pool(name="mcp", bufs=2 * IT))
        facp = ctx.enter_context(tc.tile_pool(name="facp", bufs=2 * ND2))
        ps_mm = ctx.enter_context(tc.tile_pool(name="ps_mm", bufs=4, space="PSUM"))

        ones = const.tile([1, 128], F16, tag="ones", name="ones")
        nc.vector.memset(ones[:], 1.0)

        last = None
        for rep_i in range(REPEAT):
            movs = big.tile([128, KT * MOVW], F8, tag="mv", name=f"mv{rep_i}")
            mv = movs[:].rearrange("p (k w) -> p k w", k=KT)
            sqi = sqp.tile([128, IT], F32, tag="sqi", name=f"sqi{rep_i}")
            nc.gpsimd.dma_start(sqi[:], sqi_d[:])
            sqr = srp.tile([1, MOVW], F16, tag="sqr", name=f"sqr{rep_i}")
            nc.gpsimd.dma_start(sqr[:], sqr_d[:])

            maccs = {}
            faccs = {}
            pend_drain = []   # (it, ds, psd)
            pend_tt = []      # closures

            def emit_drain(item):
                it, ds, psd = item
                is_macc_init = ds == 0
                is_facc_init = 1 <= ds <= ND2 and it == 0
                if is_macc_init:
                    to = mcp.tile([128, DW], F16, tag="macc", name=f"ma{rep_i}_{it}")
                    maccs[it] = to
                elif is_facc_init:
                    to = facp.tile([128, DW], F16, tag="facc", name=f"fa{rep_i}_{ds}")
                    faccs[ds] = to
                else:
                    to = ttp.tile([128, DW], F16, tag="to", name=f"to{rep_i}_{it}_{ds}")
                nc.scalar.activation(
                    to[:], psd[:],
                    mybir.ActivationFunctionType.Identity,
                    bias=sqi[:, it : it + 1],
                    scale=-2.0,
                )

                def tts():
                    if not is_macc_init:
                        nc.vector.tensor_tensor(
                            maccs[it][:], maccs[it][:], to[:], op=mybir.AluOpType.max
                        )
                    if 1 <= ds <= ND2 and not is_facc_init:
                        nc.vector.tensor_tensor(
                            faccs[ds][:], faccs[ds][:], to[:], op=mybir.AluOpType.max
                        )

                pend_tt.append(tts)
                while len(pend_tt) > TT_LAG:
                    pend_tt.pop(0)()

            for gi, grp in enumerate(DS_GROUPS):
                c0, c1 = grp[0] * DW, (grp[-1] + 1) * DW
                nc.gpsimd.dma_start(mv[:, :, c0:c1], mov_v[:, :, c0:c1])
                for it in range(IT):
                    psds = {
                        ds: ps_mm.tile(
                            [128, DW], F32, tag="psd", name=f"psd{rep_i}_{it}_{ds}"
                        )
                        for ds in grp
                    }
                    for ds in grp:
                        for h in range(2):
                            c = ds * DW + h * JW
                            nc.tensor.matmul(
                                psds[ds][:, h * JW : (h + 1) * JW],
                                ones[:],
                                sqr[:, c : c + JW],
                                start=True,
                                stop=False,
                            )
                    for kk in range(KK):
                        for ds in grp:
                            for h in range(2):
                                c = ds * DW + h * JW
                                nc.tensor.matmul(
                                    psds[ds][:, h * JW : (h + 1) * JW],
                                    mv[:, 2 * kk : 2 * kk + 2, it * 128 : (it + 1) * 128],
                                    mv[:, 2 * kk : 2 * kk + 2, c : c + JW],
                                    start=False,
                                    stop=(kk == KK - 1),
                                    perf_mode=mybir.MatmulPerfMode.DoubleRow,
                                )
                    for ds in grp:
                        pend_drain.append((it, ds, psds[ds]))
                    while len(pend_drain) > DRAIN_LAG:
                        emit_drain(pend_drain.pop(0))

            while pend_drain:
                emit_drain(pend_drain.pop(0))
            while pend_tt:
                pend_tt.pop(0)()
            last = (maccs, faccs)

        maccs, faccs = last
        for it in range(IT):
            nc.gpsimd.dma_start(out_m1[:, it * DW : (it + 1) * DW], maccs[it][:])
        for d in range(1, ND2 + 1):
            nc.gpsimd.dma_start(out_m2[:, (d - 1) * DW : d * DW], faccs[d][:])

    nc.compile()
    return nc


def _get_nc():
    global _NC
    if _NC is None:
        _NC = _build_nc()
    return _NC


def _make_in_maps(batch, positive):
    f32 = np.float32
    bT8 = np.ascontiguousarray(batch.T).astype(ml_dtypes.float8_e4m3)
    sq = np.einsum("nd,nd->n", batch, batch, dtype=f32)
    in_maps = []
    for c in range(NCORES):
        panels = [(c + d) % NCORES for d in range(NPAN)]
        mov = np.concatenate([bT8[:, t * OWN : (t + 1) * OWN] for t in panels], axis=1)
        sq_cols = np.concatenate([sq[t * OWN : (t + 1) * OWN] for t in panels])
        sqiT = np.ascontiguousarray(
            sq[c * OWN : (c + 1) * OWN].reshape(IT, 128).T
        ).astype(f32)
        in_maps.append(
            {
                "mov": np.ascontiguousarray(mov),
                "sqi": sqiT,
                "sqr": (-0.5 * sq_cols).astype(np.float16).reshape(1, MOVW),
            }
        )
    return in_maps


def _combine(results, batch, positive):
    f32 = np.float32
    m1 = np.concatenate(
        [
            results[c]["out_m1"]
            .astype(f32)
            .reshape(128, IT, DW)
            .max(axis=2)
            .T.reshape(-1)
            for c in range(NCORES)
        ]
    )
    d2m = m1
    for d in range(1, ND2 + 1):
        cand = np.empty(N, f32)
        for c in range(NCORES):
            tgt = (c + d) % NCORES
            cand[tgt * OWN : (tgt + 1) * OWN] = (
                results[c]["out_m2"][:, (d - 1) * DW : d * DW].astype(f32).max(axis=0)
            )
        d2m = np.maximum(d2m, cand)
    deps2 = f32(D * EPS * EPS)
    max_neg = np.sqrt(np.maximum(d2m + deps2, f32(0.0)))
    diff = batch - positive[None, :] + f32(EPS)
    pos_dist = np.sqrt(np.einsum("nd,nd->n", diff, diff, dtype=f32))
    losses = np.maximum(pos_dist - max_neg + f32(MARGIN), f32(0.0))
    valid = ~np.all(batch == positive[None, :], axis=1)
    cnt = f32(valid.sum())
    total = f32(np.sum(losses[valid], dtype=f32))
    return np.asarray(total / cnt, dtype=np.float32)


def run_on_cores(batch, positive, **kwargs):
    nc = _get_nc()
    in_maps = _make_in_maps(batch, positive)
    return bass_utils.run_bass_kernel_spmd(
        nc, in_maps, core_ids=list(range(NCORES)), **kwargs
    )


def kernel(batch, positive):
    batch = np.asarray(batch, dtype=np.float32)
    positive = np.asarray(positive, dtype=np.float32)
    res = run_on_cores(batch, positive)
    return _combine(res.results, batch, positive)
